# revision 3
# baseline (speedup 1.0000x reference)
"""KNN-impute (nn_CalcImpute) Trainium2 Bass kernel.

kernel(**inputs) takes the FULL inputs and returns the FULL output:
  dist_pot_donors [4096, 100000] f32, fit_X_col [100000] f32,
  mask_fit_X_col [100000] int, n_neighbors (=5)  ->  [4096] f32

Strategy (per the row-parallel sharding): shard rows of dist_pot_donors
across 8 NeuronCores (512 rows each); replicate the small donor vectors.

Per-core device algorithm (S=500-column subchunks, NSUB=200 per row):
  1. stream the shard once, reduce_min per subchunk -> minbuf [row, 200]
  2. vector.max (top-8) + max_index on -minbuf -> the 8 subchunks with the
     smallest mins; sort the ids ascending so later scans run in global
     column order (preserves jax.lax.top_k lowest-index tie-breaking)
  3. indirect-DMA gather those 8 subchunks of the distance row plus the
     matching slices of a host-prepared aux array [y|z] per subchunk,
     where y = fit_X * (1 - mask), z = (1 - mask)
  4. vector.max over the gathered 4000 negated distances -> top-8 values;
     sentinel all but the first K; match_replace marks exactly the K
     winning positions (first occurrence per duplicate = lowest column);
     masked sums give num = sum(y[sel]), den = sum(z[sel]);
     res = num / (den + (den == 0))
  A per-row coverage flag (8th-smallest subchunk-min <= K-th smallest
  value) marks rows whose candidate set could be incomplete under extreme
  value ties; those rows (expected: none) are recomputed exactly on host.

NaN distances (which the reference down-weights) cannot occur for this
problem's uniform-random distance matrix and are not handled on device.
"""

import sys

for _p in ("/opt/pypackages", "/opt/trn_rl_repo"):
    if _p not in sys.path:
        sys.path.insert(0, _p)

import numpy as np

import concourse.bass as bass
import concourse.bacc as bacc
import concourse.mybir as mybir
from concourse import tile
from concourse.bass import IndirectOffsetOnAxis

F32 = mybir.dt.float32
I32 = mybir.dt.int32
U32 = mybir.dt.uint32

SENTINEL = 1.0e30

N_RECV = 4096
N_DONORS = 100000
N_CORES = 8
R = N_RECV // N_CORES   # 512 rows per core
D = N_DONORS
S = 500                 # subchunk size; divides D
CT = 4000               # streaming tile cols
NG = 8                  # gathered subchunks per row (vector.max width)
NSUB = D // S
NRT = R // 128
NCT = D // CT
SPT = CT // S

# ---------------------------------------------------------------------------


def build_kernel(K: int) -> bass.Bass:
    assert 1 <= K <= 8

    nc = bacc.Bacc()
    dist = nc.dram_tensor("dist", [R * D], F32, kind="ExternalInput")
    aux = nc.dram_tensor("aux", [NSUB * 2 * S], F32, kind="ExternalInput")
    out = nc.dram_tensor("out", [R, 2], F32, kind="ExternalOutput")

    dist2d = dist[:].rearrange("(r d) -> r d", d=D)

    with tile.TileContext(nc) as tc:
        with (
            tc.tile_pool(name="const", bufs=1) as constp,
            tc.tile_pool(name="stream", bufs=3) as streamp,
            tc.tile_pool(name="minb", bufs=2) as minbp,
            tc.tile_pool(name="small", bufs=2) as smallp,
            tc.tile_pool(name="gath", bufs=2) as gathp,
        ):
            # constant: per-partition [0..7] as f32 for the rank-permute
            iota_t_i = constp.tile([128, 8], I32)
            nc.gpsimd.iota(iota_t_i[:], pattern=[[1, 8]], base=0,
                           channel_multiplier=0)
            iota_t = constp.tile([128, 8], F32)
            nc.vector.tensor_copy(iota_t[:], iota_t_i[:])

            for rt in range(NRT):
                rows = slice(rt * 128, (rt + 1) * 128)

                # ---- phase 1: streaming subchunk mins ----
                minbuf = minbp.tile([128, NSUB], F32)
                for ct in range(NCT):
                    st = streamp.tile([128, CT], F32, tag="stream")
                    nc.sync.dma_start(
                        st[:], dist2d[rows, ct * CT:(ct + 1) * CT])
                    nc.vector.tensor_reduce(
                        out=minbuf[:, ct * SPT:(ct + 1) * SPT],
                        in_=st[:].rearrange("p (a b) -> p a b", b=S),
                        axis=mybir.AxisListType.X,
                        op=mybir.AluOpType.min,
                    )

                # ---- phase 2: top-8 subchunks by min, sorted ascending ----
                negmin = smallp.tile([128, NSUB], F32, tag="negmin")
                nc.scalar.mul(negmin[:], minbuf[:], -1.0)
                m8 = smallp.tile([128, 8], F32, tag="m8")
                nc.vector.max(out=m8[:], in_=negmin[:])
                s8u = smallp.tile([128, 8], U32, tag="s8u")
                nc.vector.max_index(s8u[:], m8[:], negmin[:])
                s8f = smallp.tile([128, 8], F32, tag="s8f")
                nc.vector.tensor_copy(s8f[:], s8u[:])

                # rank_i = #{j : s[j] < s[i]} ; the ids are distinct
                cmp = smallp.tile([128, 64], F32, tag="cmp")
                cmp_v = cmp[:].rearrange("p (i j) -> p i j", j=8)
                nc.vector.tensor_tensor(
                    out=cmp_v,
                    in0=s8f[:].unsqueeze(2).to_broadcast([128, 8, 8]),
                    in1=s8f[:].unsqueeze(1).to_broadcast([128, 8, 8]),
                    op=mybir.AluOpType.is_gt,
                )
                rank = smallp.tile([128, 8], F32, tag="rank")
                nc.vector.tensor_reduce(
                    out=rank[:], in_=cmp_v, axis=mybir.AxisListType.X,
                    op=mybir.AluOpType.add)

                # sorted[t] = sum_i s[i] * [rank[i] == t]
                eq = smallp.tile([128, 64], F32, tag="eq")
                eq_v = eq[:].rearrange("p (t i) -> p t i", i=8)
                nc.vector.tensor_tensor(
                    out=eq_v,
                    in0=rank[:].unsqueeze(1).to_broadcast([128, 8, 8]),
                    in1=iota_t[:].unsqueeze(2).to_broadcast([128, 8, 8]),
                    op=mybir.AluOpType.is_equal,
                )
                nc.vector.tensor_tensor(
                    out=eq_v,
                    in0=eq_v,
                    in1=s8f[:].unsqueeze(1).to_broadcast([128, 8, 8]),
                    op=mybir.AluOpType.mult,
                )
                ssort = smallp.tile([128, 8], F32, tag="ssort")
                nc.vector.tensor_reduce(
                    out=ssort[:], in_=eq_v, axis=mybir.AxisListType.X,
                    op=mybir.AluOpType.add)

                # element offsets: idxD = row*D + s*S ; idxA = s*2S
                s_i = smallp.tile([128, 8], I32, tag="s_i")
                nc.vector.tensor_copy(s_i[:], ssort[:])
                rowbase = smallp.tile([128, 1], I32, tag="rowbase")
                nc.gpsimd.iota(rowbase[:], pattern=[[1, 1]], base=rt * 128 * D,
                               channel_multiplier=D)
                idxD = smallp.tile([128, 8], I32, tag="idxD")
                nc.vector.tensor_scalar_mul(idxD[:], s_i[:], S)
                nc.vector.tensor_tensor(
                    out=idxD[:], in0=idxD[:],
                    in1=rowbase[:].to_broadcast([128, 8]),
                    op=mybir.AluOpType.add)
                idxA = smallp.tile([128, 8], I32, tag="idxA")
                nc.vector.tensor_scalar_mul(idxA[:], s_i[:], 2 * S)

                # ---- phase 3: gather the 8 subchunks of d and aux ----
                dg = gathp.tile([128, NG * S], F32, tag="dg")
                nc.gpsimd.indirect_dma_start(
                    out=dg[:], out_offset=None,
                    in_=dist[:].unsqueeze(0),
                    in_offset=IndirectOffsetOnAxis(ap=idxD[:], axis=1),
                )
                ag = gathp.tile([128, NG * 2 * S], F32, tag="ag")
                nc.gpsimd.indirect_dma_start(
                    out=ag[:], out_offset=None,
                    in_=aux[:].unsqueeze(0),
                    in_offset=IndirectOffsetOnAxis(ap=idxA[:], axis=1),
                )

                # ---- phase 4: exact top-K + masked sums ----
                nc.scalar.mul(dg[:], dg[:], -1.0)  # dg := -d, in place
                topv = smallp.tile([128, 8], F32, tag="topv")
                nc.vector.max(out=topv[:], in_=dg[:])
                topk = smallp.tile([128, 8], F32, tag="topk")
                nc.vector.tensor_copy(topk[:], topv[:])
                if K < 8:
                    nc.vector.memset(topk[:, K:], SENTINEL)
                sel = gathp.tile([128, NG * S], F32, tag="sel")
                nc.vector.match_replace(
                    out=sel[:], in_to_replace=topk[:], in_values=dg[:],
                    imm_value=SENTINEL)
                nc.vector.tensor_scalar(
                    sel[:], sel[:], 0.5 * SENTINEL, None,
                    op0=mybir.AluOpType.is_ge)

                # ag[p, g, c, s] *= sel[p, g, s]   (c: 0=y, 1=z)
                ag_v = ag[:].rearrange("p (g c s) -> p g c s", c=2, s=S)
                sel_v = (sel[:].rearrange("p (g s) -> p g s", s=S).unsqueeze(2)
                         .to_broadcast([128, NG, 2, S]))
                nc.vector.tensor_tensor(
                    out=ag_v, in0=ag_v, in1=sel_v, op=mybir.AluOpType.mult)
                red1 = smallp.tile([128, 2 * NG], F32, tag="red1")
                nc.vector.tensor_reduce(
                    out=red1[:], in_=ag_v, axis=mybir.AxisListType.X,
                    op=mybir.AluOpType.add)
                numden = smallp.tile([128, 2], F32, tag="numden")
                nc.vector.tensor_reduce(
                    out=numden[:],
                    in_=red1[:].rearrange("p (g c) -> p c g", c=2),
                    axis=mybir.AxisListType.X,
                    op=mybir.AluOpType.add)

                # res = num / (den + (den == 0))
                eps0 = smallp.tile([128, 1], F32, tag="eps0")
                nc.vector.tensor_scalar(
                    eps0[:], numden[:, 1:2], 0.0, None,
                    op0=mybir.AluOpType.is_equal)
                den1 = smallp.tile([128, 1], F32, tag="den1")
                nc.vector.tensor_tensor(
                    out=den1[:], in0=numden[:, 1:2], in1=eps0[:],
                    op=mybir.AluOpType.add)
                rden = smallp.tile([128, 1], F32, tag="rden")
                nc.vector.reciprocal(rden[:], den1[:])

                ob = smallp.tile([128, 2], F32, tag="ob")
                nc.vector.tensor_tensor(
                    out=ob[:, 0:1], in0=numden[:, 0:1], in1=rden[:],
                    op=mybir.AluOpType.mult)
                # coverage flag (neg space): m8[:,7] >= topv[:,K-1]
                nc.vector.tensor_tensor(
                    out=ob[:, 1:2], in0=m8[:, 7:8], in1=topv[:, K - 1:K],
                    op=mybir.AluOpType.is_ge)

                nc.sync.dma_start(out[:][rows, :], ob[:])

    nc.finalize()
    return nc


_KERNEL_CACHE: dict[int, bass.Bass] = {}
LAST_RESULTS = None
PROFILE = False


def _get_kernel(K: int) -> bass.Bass:
    if K not in _KERNEL_CACHE:
        _KERNEL_CACHE[K] = build_kernel(K)
    return _KERNEL_CACHE[K]


def _host_row(d_row, y, z, K):
    order = np.argsort(d_row, kind="stable")[:K]
    num = np.float32(0.0)
    den = np.float32(0.0)
    for j in order:
        num += y[j]
        den += z[j]
    div = np.float32(1.0) if den == 0 else den
    return np.float32(num / div)


def _host_full(d, y, z, K):
    return np.array([_host_row(d[r], y, z, K) for r in range(d.shape[0])],
                    np.float32)


def kernel(dist_pot_donors, fit_X_col, mask_fit_X_col, n_neighbors):
    from concourse.bass_utils import run_bass_kernel_spmd

    global LAST_RESULTS

    d = np.ascontiguousarray(np.asarray(dist_pot_donors, dtype=np.float32))
    x = np.asarray(fit_X_col, dtype=np.float32)
    m = np.asarray(mask_fit_X_col)
    K = int(np.asarray(n_neighbors))

    z = (1 - m).astype(np.float32)
    y = x * z

    if d.shape != (N_RECV, N_DONORS) or not (1 <= K <= 8):
        return _host_full(d, y, z, K)

    aux = np.empty((NSUB, 2, S), np.float32)
    aux[:, 0, :] = y.reshape(NSUB, S)
    aux[:, 1, :] = z.reshape(NSUB, S)
    aux_flat = np.ascontiguousarray(aux.reshape(-1))

    nc = _get_kernel(K)
    in_maps = [
        {"dist": d[c * R:(c + 1) * R].reshape(-1), "aux": aux_flat}
        for c in range(N_CORES)
    ]
    LAST_RESULTS = run_bass_kernel_spmd(
        nc, in_maps, core_ids=list(range(N_CORES)), trace=PROFILE)

    res = np.empty(N_RECV, np.float32)
    for c, r in enumerate(LAST_RESULTS.results):
        ob = r["out"]
        rows = slice(c * R, (c + 1) * R)
        res[rows] = ob[:, 0]
        flagged = np.nonzero(ob[:, 1] != 0)[0]
        for fr in flagged:
            gr = c * R + int(fr)
            res[gr] = _host_row(d[gr], y, z, K)

    return res


# revision 5
# speedup vs baseline: 1.2706x; 1.2706x over previous
"""KNN-impute (nn_CalcImpute) Trainium2 Bass kernel.

kernel(**inputs) takes the FULL inputs and returns the FULL output:
  dist_pot_donors [4096, 100000] f32, fit_X_col [100000] f32,
  mask_fit_X_col [100000] int, n_neighbors (=5)  ->  [4096] f32

Strategy (row-parallel sharding): shard rows of dist_pot_donors across
8 NeuronCores (512 rows each); replicate the small donor vectors.

Per-core device algorithm (S=500-column subchunks, NSUB=200 per row):
  1. stream the shard once, reduce_min per subchunk -> minbuf [row, 200]
  2. vector.max (top-8) + max_index on -minbuf -> the 8 subchunks with the
     smallest mins; sort the ids ascending so later scans run in global
     column order (preserves jax.lax.top_k lowest-index tie-breaking)
  3. indirect-DMA gather those 8 subchunks of the distance row
  4. vector.max over the gathered 4000 negated values -> top-8 values;
     max_index -> their positions (first occurrence per duplicate =
     lowest column); decompose position -> (window, offset) -> global
     column j; indirect-DMA gather the interleaved (y, z) pair per
     winner, where y = fit_X * (1 - mask), z = (1 - mask);
     num = sum over first K winners of y, den likewise of z;
     res = num / (den + (den == 0))
  A per-row coverage flag (8th-smallest subchunk-min <= K-th smallest
  value) marks rows whose candidate set could be incomplete under extreme
  value ties; those rows (expected: none) are recomputed exactly on host.

NaN distances (which the reference down-weights) cannot occur for this
problem's uniform-random distance matrix and are not handled on device.
"""

import sys

for _p in ("/opt/pypackages", "/opt/trn_rl_repo"):
    if _p not in sys.path:
        sys.path.insert(0, _p)

import numpy as np

import concourse.bass as bass
import concourse.bacc as bacc
import concourse.mybir as mybir
from concourse import tile
from concourse.bass import IndirectOffsetOnAxis

F32 = mybir.dt.float32
I32 = mybir.dt.int32
U32 = mybir.dt.uint32

N_RECV = 4096
N_DONORS = 100000
N_CORES = 8
R = N_RECV // N_CORES   # 512 rows per core
D = N_DONORS
S = 500                 # subchunk size; divides D
CT = 10000              # streaming tile cols; multiple of S, divides D
NG = 8                  # gathered subchunks per row (vector.max width)


def build_kernel(K: int, R: int = R, D: int = D, S: int = S,
                 CT: int = CT) -> bass.Bass:
    NSUB = D // S
    NRT = R // 128
    NCT = D // CT
    SPT = CT // S
    assert D % S == 0 and D % CT == 0 and CT % S == 0
    assert R % 128 == 0 and 1 <= K <= 8
    assert 8 <= NSUB <= 16384 and 8 <= NG * S <= 16384

    nc = bacc.Bacc()
    dist = nc.dram_tensor("dist", [R * D], F32, kind="ExternalInput")
    # auxyz[2j] = y[j] = x[j]*(1-m[j]); auxyz[2j+1] = z[j] = 1-m[j]
    auxyz = nc.dram_tensor("auxyz", [2 * D], F32, kind="ExternalInput")
    out = nc.dram_tensor("out", [R, 2], F32, kind="ExternalOutput")

    dist2d = dist[:].rearrange("(r d) -> r d", d=D)

    with tile.TileContext(nc) as tc:
        with (
            tc.tile_pool(name="const", bufs=1) as constp,
            tc.tile_pool(name="stream", bufs=3) as streamp,
            tc.tile_pool(name="minb", bufs=2) as minbp,
            tc.tile_pool(name="small", bufs=2) as smallp,
            tc.tile_pool(name="gath", bufs=2) as gathp,
        ):
            # constants: per-partition [0..7] and window thresholds
            iota_t_i = constp.tile([128, 8], I32)
            nc.gpsimd.iota(iota_t_i[:], pattern=[[1, 8]], base=0,
                           channel_multiplier=0)
            iota_t = constp.tile([128, 8], F32)
            nc.vector.tensor_copy(iota_t[:], iota_t_i[:])
            thr_i = constp.tile([128, 7], I32)
            nc.gpsimd.iota(thr_i[:], pattern=[[S, 7]], base=S,
                           channel_multiplier=0)
            thr = constp.tile([128, 7], F32)
            nc.vector.tensor_copy(thr[:], thr_i[:])

            for rt in range(NRT):
                rows = slice(rt * 128, (rt + 1) * 128)

                # ---- phase 1: streaming subchunk mins ----
                minbuf = minbp.tile([128, NSUB], F32)
                for ct in range(NCT):
                    st = streamp.tile([128, CT], F32, tag="stream")
                    nc.sync.dma_start(
                        st[:], dist2d[rows, ct * CT:(ct + 1) * CT])
                    nc.vector.tensor_reduce(
                        out=minbuf[:, ct * SPT:(ct + 1) * SPT],
                        in_=st[:].rearrange("p (a b) -> p a b", b=S),
                        axis=mybir.AxisListType.X,
                        op=mybir.AluOpType.min,
                    )

                # ---- phase 2: top-8 subchunks by min, sorted ascending ----
                negmin = smallp.tile([128, NSUB], F32, tag="negmin")
                nc.scalar.mul(negmin[:], minbuf[:], -1.0)
                m8 = smallp.tile([128, 8], F32, tag="m8")
                nc.vector.max(out=m8[:], in_=negmin[:])
                s8u = smallp.tile([128, 8], U32, tag="s8u")
                nc.vector.max_index(s8u[:], m8[:], negmin[:])
                s8f = smallp.tile([128, 8], F32, tag="s8f")
                nc.vector.tensor_copy(s8f[:], s8u[:])

                # rank_i = #{j : s[j] < s[i]} ; the ids are distinct
                cmp = smallp.tile([128, 64], F32, tag="cmp")
                cmp_v = cmp[:].rearrange("p (i j) -> p i j", j=8)
                nc.vector.tensor_tensor(
                    out=cmp_v,
                    in0=s8f[:].unsqueeze(2).to_broadcast([128, 8, 8]),
                    in1=s8f[:].unsqueeze(1).to_broadcast([128, 8, 8]),
                    op=mybir.AluOpType.is_gt,
                )
                rank = smallp.tile([128, 8], F32, tag="rank")
                nc.vector.tensor_reduce(
                    out=rank[:], in_=cmp_v, axis=mybir.AxisListType.X,
                    op=mybir.AluOpType.add)

                # ssort[t] = sum_i s[i] * [rank[i] == t]
                eq = smallp.tile([128, 64], F32, tag="eq")
                eq_v = eq[:].rearrange("p (t i) -> p t i", i=8)
                nc.vector.tensor_tensor(
                    out=eq_v,
                    in0=rank[:].unsqueeze(1).to_broadcast([128, 8, 8]),
                    in1=iota_t[:].unsqueeze(2).to_broadcast([128, 8, 8]),
                    op=mybir.AluOpType.is_equal,
                )
                nc.vector.tensor_tensor(
                    out=eq_v,
                    in0=eq_v,
                    in1=s8f[:].unsqueeze(1).to_broadcast([128, 8, 8]),
                    op=mybir.AluOpType.mult,
                )
                ssort = smallp.tile([128, 8], F32, tag="ssort")
                nc.vector.tensor_reduce(
                    out=ssort[:], in_=eq_v, axis=mybir.AxisListType.X,
                    op=mybir.AluOpType.add)

                # element offsets into dist: idxD = row*D + s*S
                s_i = smallp.tile([128, 8], I32, tag="s_i")
                nc.vector.tensor_copy(s_i[:], ssort[:])
                rowbase = smallp.tile([128, 1], I32, tag="rowbase")
                nc.gpsimd.iota(rowbase[:], pattern=[[1, 1]], base=rt * 128 * D,
                               channel_multiplier=D)
                idxD = smallp.tile([128, 8], I32, tag="idxD")
                nc.vector.tensor_scalar_mul(idxD[:], s_i[:], S)
                nc.vector.tensor_tensor(
                    out=idxD[:], in0=idxD[:],
                    in1=rowbase[:].to_broadcast([128, 8]),
                    op=mybir.AluOpType.add)

                # ---- phase 3: gather the 8 subchunks of d ----
                dg = gathp.tile([128, NG * S], F32, tag="dg")
                nc.gpsimd.indirect_dma_start(
                    out=dg[:], out_offset=None,
                    in_=dist[:].unsqueeze(0),
                    in_offset=IndirectOffsetOnAxis(ap=idxD[:], axis=1),
                )

                # ---- phase 4: top-8 values + positions ----
                nc.scalar.mul(dg[:], dg[:], -1.0)  # dg := -d, in place
                topv = smallp.tile([128, 8], F32, tag="topv")
                nc.vector.max(out=topv[:], in_=dg[:])
                topp_u = smallp.tile([128, 8], U32, tag="topp_u")
                nc.vector.max_index(topp_u[:], topv[:], dg[:])
                topp = smallp.tile([128, 8], F32, tag="topp")
                nc.vector.tensor_copy(topp[:], topp_u[:])

                # wrank_i = which window slot position i falls in (0..7)
                wcmp = smallp.tile([128, 56], F32, tag="wcmp")
                wcmp_v = wcmp[:].rearrange("p (i t) -> p i t", t=7)
                nc.vector.tensor_tensor(
                    out=wcmp_v,
                    in0=topp[:].unsqueeze(2).to_broadcast([128, 8, 7]),
                    in1=thr[:].unsqueeze(1).to_broadcast([128, 8, 7]),
                    op=mybir.AluOpType.is_ge,
                )
                wrank = smallp.tile([128, 8], F32, tag="wrank")
                nc.vector.tensor_reduce(
                    out=wrank[:], in_=wcmp_v, axis=mybir.AxisListType.X,
                    op=mybir.AluOpType.add)

                # pos = topp - wrank*S ; s_at[i] = ssort[wrank_i]
                pos = smallp.tile([128, 8], F32, tag="pos")
                nc.vector.tensor_scalar_mul(pos[:], wrank[:], -float(S))
                nc.vector.tensor_tensor(
                    out=pos[:], in0=pos[:], in1=topp[:],
                    op=mybir.AluOpType.add)
                weq = smallp.tile([128, 64], F32, tag="weq")
                weq_v = weq[:].rearrange("p (i t) -> p i t", t=8)
                nc.vector.tensor_tensor(
                    out=weq_v,
                    in0=wrank[:].unsqueeze(2).to_broadcast([128, 8, 8]),
                    in1=iota_t[:].unsqueeze(1).to_broadcast([128, 8, 8]),
                    op=mybir.AluOpType.is_equal,
                )
                nc.vector.tensor_tensor(
                    out=weq_v,
                    in0=weq_v,
                    in1=ssort[:].unsqueeze(1).to_broadcast([128, 8, 8]),
                    op=mybir.AluOpType.mult,
                )
                s_at = smallp.tile([128, 8], F32, tag="s_at")
                nc.vector.tensor_reduce(
                    out=s_at[:], in_=weq_v, axis=mybir.AxisListType.X,
                    op=mybir.AluOpType.add)

                # idxYZ = 2*(s_at*S + pos)   (exact in f32: < 2^24)
                idxYZf = smallp.tile([128, 8], F32, tag="idxYZf")
                nc.vector.tensor_scalar_mul(idxYZf[:], s_at[:], float(2 * S))
                nc.vector.tensor_scalar_mul(pos[:], pos[:], 2.0)
                nc.vector.tensor_tensor(
                    out=idxYZf[:], in0=idxYZf[:], in1=pos[:],
                    op=mybir.AluOpType.add)
                idxYZ = smallp.tile([128, 8], I32, tag="idxYZ")
                nc.vector.tensor_copy(idxYZ[:], idxYZf[:])

                yz = smallp.tile([128, 16], F32, tag="yz")
                nc.gpsimd.indirect_dma_start(
                    out=yz[:], out_offset=None,
                    in_=auxyz[:].unsqueeze(0),
                    in_offset=IndirectOffsetOnAxis(ap=idxYZ[:], axis=1),
                )

                # num/den = sums over the first K winners
                yz_v = yz[:].rearrange("p (i c) -> p c i", c=2)  # strided view
                numden = smallp.tile([128, 2], F32, tag="numden")
                nc.vector.tensor_reduce(
                    out=numden[:], in_=yz_v[:, :, 0:K],
                    axis=mybir.AxisListType.X, op=mybir.AluOpType.add)

                # res = num / (den + (den == 0))
                eps0 = smallp.tile([128, 1], F32, tag="eps0")
                nc.vector.tensor_scalar(
                    eps0[:], numden[:, 1:2], 0.0, None,
                    op0=mybir.AluOpType.is_equal)
                den1 = smallp.tile([128, 1], F32, tag="den1")
                nc.vector.tensor_tensor(
                    out=den1[:], in0=numden[:, 1:2], in1=eps0[:],
                    op=mybir.AluOpType.add)
                rden = smallp.tile([128, 1], F32, tag="rden")
                nc.vector.reciprocal(rden[:], den1[:])

                ob = smallp.tile([128, 2], F32, tag="ob")
                nc.vector.tensor_tensor(
                    out=ob[:, 0:1], in0=numden[:, 0:1], in1=rden[:],
                    op=mybir.AluOpType.mult)
                # coverage flag (neg space): m8[:,7] >= topv[:,K-1]
                nc.vector.tensor_tensor(
                    out=ob[:, 1:2], in0=m8[:, 7:8], in1=topv[:, K - 1:K],
                    op=mybir.AluOpType.is_ge)

                nc.scalar.dma_start(out[:][rows, :], ob[:])

    nc.finalize()
    return nc


_KERNEL_CACHE: dict[int, bass.Bass] = {}
LAST_RESULTS = None
PROFILE = False


def _get_kernel(K: int) -> bass.Bass:
    if K not in _KERNEL_CACHE:
        _KERNEL_CACHE[K] = build_kernel(K)
    return _KERNEL_CACHE[K]


def _host_row(d_row, y, z, K):
    order = np.argsort(d_row, kind="stable")[:K]
    num = np.float32(0.0)
    den = np.float32(0.0)
    for j in order:
        num += y[j]
        den += z[j]
    div = np.float32(1.0) if den == 0 else den
    return np.float32(num / div)


def _host_full(d, y, z, K):
    return np.array([_host_row(d[r], y, z, K) for r in range(d.shape[0])],
                    np.float32)


def kernel(dist_pot_donors, fit_X_col, mask_fit_X_col, n_neighbors):
    from concourse.bass_utils import run_bass_kernel_spmd

    global LAST_RESULTS

    d = np.ascontiguousarray(np.asarray(dist_pot_donors, dtype=np.float32))
    x = np.asarray(fit_X_col, dtype=np.float32)
    m = np.asarray(mask_fit_X_col)
    K = int(np.asarray(n_neighbors))

    z = (1 - m).astype(np.float32)
    y = x * z

    if d.shape != (N_RECV, N_DONORS) or not (1 <= K <= 8):
        return _host_full(d, y, z, K)

    auxyz = np.empty((D, 2), np.float32)
    auxyz[:, 0] = y
    auxyz[:, 1] = z
    auxyz_flat = np.ascontiguousarray(auxyz.reshape(-1))

    nc = _get_kernel(K)
    in_maps = [
        {"dist": d[c * R:(c + 1) * R].reshape(-1), "auxyz": auxyz_flat}
        for c in range(N_CORES)
    ]
    LAST_RESULTS = run_bass_kernel_spmd(
        nc, in_maps, core_ids=list(range(N_CORES)), trace=PROFILE)

    res = np.empty(N_RECV, np.float32)
    for c, r in enumerate(LAST_RESULTS.results):
        ob = r["out"]
        rows = slice(c * R, (c + 1) * R)
        res[rows] = ob[:, 0]
        flagged = np.nonzero(ob[:, 1] != 0)[0]
        for fr in flagged:
            gr = c * R + int(fr)
            res[gr] = _host_row(d[gr], y, z, K)

    return res


# revision 7
# speedup vs baseline: 1.3696x; 1.0780x over previous
"""KNN-impute (nn_CalcImpute) Trainium2 Bass kernel.

kernel(**inputs) takes the FULL inputs and returns the FULL output:
  dist_pot_donors [4096, 100000] f32, fit_X_col [100000] f32,
  mask_fit_X_col [100000] int, n_neighbors (=5)  ->  [4096] f32

Strategy (row-parallel sharding): shard rows of dist_pot_donors across
8 NeuronCores (512 rows each); replicate the small donor vectors.

Per-core device algorithm (S=500-column subchunks, NSUB=200 per row):
  1. stream the shard once, reduce_min per subchunk -> minbuf [row, 200]
  2. vector.max (top-8) + max_index on -minbuf -> the NG=6 subchunks with
     the smallest mins; sort the ids ascending so later scans run in
     global column order (preserves jax.lax.top_k lowest-index ties)
  3. indirect-DMA gather those NG subchunks of the distance row
  4. vector.max over the gathered NG*S negated values -> top-8 values;
     max_index -> their positions (first occurrence per duplicate =
     lowest column); decompose position -> (window, offset) -> global
     column j; indirect-DMA gather the interleaved (y, z) pair for the
     first K winners, where y = fit_X * (1 - mask), z = (1 - mask);
     num = sum(y), den = sum(z); res = num / (den + (den == 0))
  Coverage flag: a subchunk outside the gathered NG has min >= the
  (NG+1)-th smallest subchunk min; if that is <= the K-th smallest value
  the candidate set could be incomplete under value ties -> recompute the
  row exactly on host (expected: ~0 rows).

Phases 2-4 of row-tile t are emitted interleaved into row-tile t+1's
streaming so the in-order engines never stall on the gather latency.

NaN distances (which the reference down-weights) cannot occur for this
problem's uniform-random distance matrix and are not handled on device.
"""

import sys

for _p in ("/opt/pypackages", "/opt/trn_rl_repo"):
    if _p not in sys.path:
        sys.path.insert(0, _p)

import numpy as np

import concourse.bass as bass
import concourse.bacc as bacc
import concourse.mybir as mybir
from concourse import tile
from concourse.bass import IndirectOffsetOnAxis

F32 = mybir.dt.float32
I32 = mybir.dt.int32
U32 = mybir.dt.uint32

N_RECV = 4096
N_DONORS = 100000
N_CORES = 8
R = N_RECV // N_CORES   # 512 rows per core
D = N_DONORS
S = 500                 # subchunk size; divides D
CT = 5000               # streaming tile cols; multiple of S, divides D
NG = 6                  # gathered subchunks per row (<= 7)


def build_kernel(K: int, R: int = R, D: int = D, S: int = S,
                 CT: int = CT, NG: int = NG) -> bass.Bass:
    NSUB = D // S
    NRT = R // 128
    NCT = D // CT
    SPT = CT // S
    assert D % S == 0 and D % CT == 0 and CT % S == 0
    assert R % 128 == 0 and 1 <= K <= 8 and 2 <= NG <= 7
    assert 8 <= NSUB <= 16384 and 8 <= NG * S <= 16384

    nc = bacc.Bacc()
    dist = nc.dram_tensor("dist", [R * D], F32, kind="ExternalInput")
    # auxyz[2j] = y[j] = x[j]*(1-m[j]); auxyz[2j+1] = z[j] = 1-m[j]
    auxyz = nc.dram_tensor("auxyz", [2 * D], F32, kind="ExternalInput")
    out = nc.dram_tensor("out", [R, 2], F32, kind="ExternalOutput")

    dist2d = dist[:].rearrange("(r d) -> r d", d=D)

    with tile.TileContext(nc) as tc:
        with (
            tc.tile_pool(name="const", bufs=1) as constp,
            tc.tile_pool(name="stream", bufs=5) as streamp,
            tc.tile_pool(name="minb", bufs=2) as minbp,
            tc.tile_pool(name="small", bufs=2) as smallp,
            tc.tile_pool(name="gath", bufs=2) as gathp,
        ):
            # constants: per-partition iotas and window thresholds
            iota_g_i = constp.tile([128, NG], I32)
            nc.gpsimd.iota(iota_g_i[:], pattern=[[1, NG]], base=0,
                           channel_multiplier=0)
            iota_g = constp.tile([128, NG], F32)
            nc.vector.tensor_copy(iota_g[:], iota_g_i[:])
            thr_i = constp.tile([128, NG - 1], I32)
            nc.gpsimd.iota(thr_i[:], pattern=[[S, NG - 1]], base=S,
                           channel_multiplier=0)
            thr = constp.tile([128, NG - 1], F32)
            nc.vector.tensor_copy(thr[:], thr_i[:])

            def emit_p23(st):
                """top-NG subchunks by min (sorted ascending) + d gather."""
                rt, minbuf = st["rt"], st["minbuf"]
                negmin = smallp.tile([128, NSUB], F32, tag="negmin")
                nc.scalar.mul(negmin[:], minbuf[:], -1.0)
                m8 = smallp.tile([128, 8], F32, tag="m8")
                nc.vector.max(out=m8[:], in_=negmin[:])
                s8u = smallp.tile([128, 8], U32, tag="s8u")
                nc.vector.max_index(s8u[:], m8[:], negmin[:])
                s8f = smallp.tile([128, 8], F32, tag="s8f")
                nc.vector.tensor_copy(s8f[:], s8u[:])
                sg = s8f[:, :NG]

                # rank_i = #{j < NG : s[j] < s[i]} ; the ids are distinct
                cmp = smallp.tile([128, NG * NG], F32, tag="cmp")
                cmp_v = cmp[:].rearrange("p (i j) -> p i j", j=NG)
                nc.vector.tensor_tensor(
                    out=cmp_v,
                    in0=sg.unsqueeze(2).to_broadcast([128, NG, NG]),
                    in1=sg.unsqueeze(1).to_broadcast([128, NG, NG]),
                    op=mybir.AluOpType.is_gt,
                )
                rank = smallp.tile([128, NG], F32, tag="rank")
                nc.vector.tensor_reduce(
                    out=rank[:], in_=cmp_v, axis=mybir.AxisListType.X,
                    op=mybir.AluOpType.add)

                # ssort[t] = sum_i s[i] * [rank[i] == t]
                eq = smallp.tile([128, NG * NG], F32, tag="eq")
                eq_v = eq[:].rearrange("p (t i) -> p t i", i=NG)
                nc.vector.tensor_tensor(
                    out=eq_v,
                    in0=rank[:].unsqueeze(1).to_broadcast([128, NG, NG]),
                    in1=iota_g[:].unsqueeze(2).to_broadcast([128, NG, NG]),
                    op=mybir.AluOpType.is_equal,
                )
                nc.vector.tensor_tensor(
                    out=eq_v,
                    in0=eq_v,
                    in1=sg.unsqueeze(1).to_broadcast([128, NG, NG]),
                    op=mybir.AluOpType.mult,
                )
                ssort = smallp.tile([128, NG], F32, tag="ssort")
                nc.vector.tensor_reduce(
                    out=ssort[:], in_=eq_v, axis=mybir.AxisListType.X,
                    op=mybir.AluOpType.add)

                # element offsets into dist: idxD = row*D + s*S
                s_i = smallp.tile([128, NG], I32, tag="s_i")
                nc.vector.tensor_copy(s_i[:], ssort[:])
                rowbase = smallp.tile([128, 1], I32, tag="rowbase")
                nc.gpsimd.iota(rowbase[:], pattern=[[1, 1]],
                               base=rt * 128 * D, channel_multiplier=D)
                idxD = smallp.tile([128, NG], I32, tag="idxD")
                nc.vector.tensor_scalar_mul(idxD[:], s_i[:], S)
                nc.vector.tensor_tensor(
                    out=idxD[:], in0=idxD[:],
                    in1=rowbase[:].to_broadcast([128, NG]),
                    op=mybir.AluOpType.add)

                dg = gathp.tile([128, NG * S], F32, tag="dg")
                nc.gpsimd.indirect_dma_start(
                    out=dg[:], out_offset=None,
                    in_=dist[:].unsqueeze(0),
                    in_offset=IndirectOffsetOnAxis(ap=idxD[:], axis=1),
                )
                st.update(m8=m8, ssort=ssort, dg=dg)

            def emit_p4a(st):
                """top-8 values + positions -> (y,z) gather for K winners."""
                dg, ssort = st["dg"], st["ssort"]
                nc.scalar.mul(dg[:], dg[:], -1.0)  # dg := -d, in place
                topv = smallp.tile([128, 8], F32, tag="topv")
                nc.vector.max(out=topv[:], in_=dg[:])
                topp_u = smallp.tile([128, 8], U32, tag="topp_u")
                nc.vector.max_index(topp_u[:], topv[:], dg[:])
                topp = smallp.tile([128, 8], F32, tag="topp")
                nc.vector.tensor_copy(topp[:], topp_u[:])

                # wrank_i = which window slot position i falls in (0..NG-1)
                wcmp = smallp.tile([128, 8 * (NG - 1)], F32, tag="wcmp")
                wcmp_v = wcmp[:].rearrange("p (i t) -> p i t", t=NG - 1)
                nc.vector.tensor_tensor(
                    out=wcmp_v,
                    in0=topp[:].unsqueeze(2).to_broadcast([128, 8, NG - 1]),
                    in1=thr[:].unsqueeze(1).to_broadcast([128, 8, NG - 1]),
                    op=mybir.AluOpType.is_ge,
                )
                wrank = smallp.tile([128, 8], F32, tag="wrank")
                nc.vector.tensor_reduce(
                    out=wrank[:], in_=wcmp_v, axis=mybir.AxisListType.X,
                    op=mybir.AluOpType.add)

                # pos = topp - wrank*S ; s_at[i] = ssort[wrank_i]
                pos = smallp.tile([128, 8], F32, tag="pos")
                nc.vector.tensor_scalar_mul(pos[:], wrank[:], -float(S))
                nc.vector.tensor_tensor(
                    out=pos[:], in0=pos[:], in1=topp[:],
                    op=mybir.AluOpType.add)
                weq = smallp.tile([128, 8 * NG], F32, tag="weq")
                weq_v = weq[:].rearrange("p (i t) -> p i t", t=NG)
                nc.vector.tensor_tensor(
                    out=weq_v,
                    in0=wrank[:].unsqueeze(2).to_broadcast([128, 8, NG]),
                    in1=iota_g[:].unsqueeze(1).to_broadcast([128, 8, NG]),
                    op=mybir.AluOpType.is_equal,
                )
                nc.vector.tensor_tensor(
                    out=weq_v,
                    in0=weq_v,
                    in1=ssort[:].unsqueeze(1).to_broadcast([128, 8, NG]),
                    op=mybir.AluOpType.mult,
                )
                s_at = smallp.tile([128, 8], F32, tag="s_at")
                nc.vector.tensor_reduce(
                    out=s_at[:], in_=weq_v, axis=mybir.AxisListType.X,
                    op=mybir.AluOpType.add)

                # idxYZ = 2*(s_at*S + pos)   (exact in f32: < 2^24)
                idxYZf = smallp.tile([128, 8], F32, tag="idxYZf")
                nc.vector.tensor_scalar_mul(idxYZf[:], s_at[:], float(2 * S))
                nc.vector.tensor_scalar_mul(pos[:], pos[:], 2.0)
                nc.vector.tensor_tensor(
                    out=idxYZf[:], in0=idxYZf[:], in1=pos[:],
                    op=mybir.AluOpType.add)
                idxYZ = smallp.tile([128, 8], I32, tag="idxYZ")
                nc.vector.tensor_copy(idxYZ[:], idxYZf[:])

                yz = smallp.tile([128, 2 * K], F32, tag="yz")
                nc.gpsimd.indirect_dma_start(
                    out=yz[:], out_offset=None,
                    in_=auxyz[:].unsqueeze(0),
                    in_offset=IndirectOffsetOnAxis(ap=idxYZ[:, :K], axis=1),
                )
                st.update(topv=topv, yz=yz)

            def emit_p4b(st):
                """num/den sums, divide, flag, output DMA."""
                rt, m8, topv, yz = st["rt"], st["m8"], st["topv"], st["yz"]
                yz_v = yz[:].rearrange("p (i c) -> p c i", c=2)
                numden = smallp.tile([128, 2], F32, tag="numden")
                nc.vector.tensor_reduce(
                    out=numden[:], in_=yz_v,
                    axis=mybir.AxisListType.X, op=mybir.AluOpType.add)

                eps0 = smallp.tile([128, 1], F32, tag="eps0")
                nc.vector.tensor_scalar(
                    eps0[:], numden[:, 1:2], 0.0, None,
                    op0=mybir.AluOpType.is_equal)
                den1 = smallp.tile([128, 1], F32, tag="den1")
                nc.vector.tensor_tensor(
                    out=den1[:], in0=numden[:, 1:2], in1=eps0[:],
                    op=mybir.AluOpType.add)
                rden = smallp.tile([128, 1], F32, tag="rden")
                nc.vector.reciprocal(rden[:], den1[:])

                ob = smallp.tile([128, 2], F32, tag="ob")
                nc.vector.tensor_tensor(
                    out=ob[:, 0:1], in0=numden[:, 0:1], in1=rden[:],
                    op=mybir.AluOpType.mult)
                # coverage flag (neg space): the (NG+1)-th smallest subchunk
                # min (slot NG of m8) >= K-th smallest value
                nc.vector.tensor_tensor(
                    out=ob[:, 1:2], in0=m8[:, NG:NG + 1],
                    in1=topv[:, K - 1:K],
                    op=mybir.AluOpType.is_ge)

                rows = slice(rt * 128, (rt + 1) * 128)
                nc.scalar.dma_start(out[:][rows, :], ob[:])

            # interleave points inside the NEXT row-tile's streaming
            i23 = 1
            i4a = max(2, min(NCT - 2, NCT // 2))
            i4b = NCT - 1

            pending = None
            for rt in range(NRT):
                minbuf = minbp.tile([128, NSUB], F32)
                sched = ([] if pending is None else
                         [(i23, emit_p23), (i4a, emit_p4a), (i4b, emit_p4b)])
                for ct in range(NCT):
                    st_t = streamp.tile([128, CT], F32, tag="stream")
                    nc.sync.dma_start(
                        st_t[:], dist2d[rt * 128:(rt + 1) * 128,
                                        ct * CT:(ct + 1) * CT])
                    nc.vector.tensor_reduce(
                        out=minbuf[:, ct * SPT:(ct + 1) * SPT],
                        in_=st_t[:].rearrange("p (a b) -> p a b", b=S),
                        axis=mybir.AxisListType.X,
                        op=mybir.AluOpType.min,
                    )
                    while sched and ct >= sched[0][0]:
                        sched.pop(0)[1](pending)
                while sched:
                    sched.pop(0)[1](pending)
                pending = {"rt": rt, "minbuf": minbuf}

            emit_p23(pending)
            emit_p4a(pending)
            emit_p4b(pending)

    nc.finalize()
    return nc


_KERNEL_CACHE: dict[int, bass.Bass] = {}
LAST_RESULTS = None
PROFILE = False


def _get_kernel(K: int) -> bass.Bass:
    if K not in _KERNEL_CACHE:
        _KERNEL_CACHE[K] = build_kernel(K)
    return _KERNEL_CACHE[K]


def _host_row(d_row, y, z, K):
    order = np.argsort(d_row, kind="stable")[:K]
    num = np.float32(0.0)
    den = np.float32(0.0)
    for j in order:
        num += y[j]
        den += z[j]
    div = np.float32(1.0) if den == 0 else den
    return np.float32(num / div)


def _host_full(d, y, z, K):
    return np.array([_host_row(d[r], y, z, K) for r in range(d.shape[0])],
                    np.float32)


def kernel(dist_pot_donors, fit_X_col, mask_fit_X_col, n_neighbors):
    from concourse.bass_utils import run_bass_kernel_spmd

    global LAST_RESULTS

    d = np.ascontiguousarray(np.asarray(dist_pot_donors, dtype=np.float32))
    x = np.asarray(fit_X_col, dtype=np.float32)
    m = np.asarray(mask_fit_X_col)
    K = int(np.asarray(n_neighbors))

    z = (1 - m).astype(np.float32)
    y = x * z

    if d.shape != (N_RECV, N_DONORS) or not (1 <= K <= 8):
        return _host_full(d, y, z, K)

    auxyz = np.empty((D, 2), np.float32)
    auxyz[:, 0] = y
    auxyz[:, 1] = z
    auxyz_flat = np.ascontiguousarray(auxyz.reshape(-1))

    nc = _get_kernel(K)
    in_maps = [
        {"dist": d[c * R:(c + 1) * R].reshape(-1), "auxyz": auxyz_flat}
        for c in range(N_CORES)
    ]
    LAST_RESULTS = run_bass_kernel_spmd(
        nc, in_maps, core_ids=list(range(N_CORES)), trace=PROFILE)

    res = np.empty(N_RECV, np.float32)
    for c, r in enumerate(LAST_RESULTS.results):
        ob = r["out"]
        rows = slice(c * R, (c + 1) * R)
        res[rows] = ob[:, 0]
        flagged = np.nonzero(ob[:, 1] != 0)[0]
        for fr in flagged:
            gr = c * R + int(fr)
            res[gr] = _host_row(d[gr], y, z, K)

    return res
